# revision 4
# baseline (speedup 1.0000x reference)
"""BitLinear (RMSNorm + ternary-quantized linear) on 8 TRN2 NeuronCores, v3.

Sharding: data-parallel over tokens (B*S = 8192 -> 1024 per core), weight
replicated. gamma = mean(|w|) computed locally per core.

v3 findings baked in (from v1=294us, v2=270us traces):
  - per-HWDGE-queue DMA is ~150-190GB/s; aggregate HBM ~390GB/s/core.
    DGE queues fan out over 16 engines with NO strict FIFO priority, so
    "queueing x behind pass-1" only partially prioritizes pass-1. gamma
    lands ~54us (16MB at HBM rate + 9us startup) -- that is the floor.
  - f32 PE transposes cost ~2.5x bf16: x is cast to bf16 on ACT first
    (the rmsnorm multiply is still folded into the drain: the per-token
    scale gamma/rms lands in the PSUM-drain tensor_scalar).
  - quantize groups run in order G0=d12..15, G1=d8..11, G2=d0..3,
    G3=d4..7: the last-arrived pass-1 blocks are still resident in the
    wf pool (bufs=8) when gamma lands, so quantize starts immediately
    and only d0..d7 (8MB) is re-read -- timed to land right when the
    rotation frees buffers, well before G2/G3 need it.
  - wqt stored as 4 k-group tiles so matmuls can start before the last
    transpose copies finish.

Engine notes from profiling this HW path (keep):
  - gpsimd tensor_scalar and DVE scalar_tensor_tensor run 24-31us per
    [128,2048] tile -- avoid; single-op DVE tensor_scalar is ~1-2us.
  - InstTensorTensorReduce crashes the device; ACT Square+accum_out works.
  - Fused two-op tensor_scalar with an AP scalar in op1 fails ISA checks
    (imm op1 scalars are fine).
  - DMA x-bar transpose corrupts under concurrency -- PE transposes only.
"""

import os
import sys

for _p in ("/opt/trn_rl_repo",):
    if _p not in sys.path:
        sys.path.insert(0, _p)

import numpy as np

import concourse.bacc as bacc
import concourse.tile as tile
import concourse.mybir as mybir
from concourse import masks
from concourse.bass_utils import run_bass_kernel_spmd

NORM_EPS = 1e-6
QUANT_EPS = 1e-8

B, S, DIN, DOUT = 2, 4096, 2048, 2048
NCORES = 8
TOKS = B * S              # 8192 total tokens
TOK = TOKS // NCORES      # 1024 tokens per core
TT = TOK // 128           # 8 token tiles per core
KC = DIN // 128           # 16 contraction chunks
WB = DOUT // 16           # -> 128 rows per weight block
WB = 16                   # 16 weight row blocks
NG = 4                    # output column groups (512 cols each)
JB = 4                    # weight blocks per group
KG = 4                    # k-chunks per wqt tile
# group -> weight blocks (first two groups are pass-1-resident)
GROUPS = [[12, 13, 14, 15], [8, 9, 10, 11], [0, 1, 2, 3], [4, 5, 6, 7]]

F32 = mybir.dt.float32
BF16 = mybir.dt.bfloat16
ALU = mybir.AluOpType
ACTF = mybir.ActivationFunctionType


def _build():
    nc = bacc.Bacc(
        "TRN2", target_bir_lowering=False, debug=False, num_devices=NCORES
    )

    x_d = nc.dram_tensor("x", [TOK, DIN], F32, kind="ExternalInput")
    w_d = nc.dram_tensor("weight", [DOUT, DIN], F32, kind="ExternalInput")
    nw_d = nc.dram_tensor("norm_weight", [DIN], F32, kind="ExternalInput")
    out_d = nc.dram_tensor("out", [TOK, DOUT], F32, kind="ExternalOutput")

    with tile.TileContext(nc) as tc:
        with (
            tc.tile_pool(name="const", bufs=1) as const,
            tc.tile_pool(name="spool", bufs=4) as spool,
            tc.tile_pool(name="gpool", bufs=1) as gpool,
            tc.tile_pool(name="xin", bufs=2) as xin,
            tc.tile_pool(name="xbf", bufs=2) as xbf,
            tc.tile_pool(name="xntp", bufs=TT) as xntp,
            tc.tile_pool(name="wf", bufs=8) as wf,
            tc.tile_pool(name="wmp", bufs=2) as wmp,
            tc.tile_pool(name="wqp", bufs=4) as wqp,
            tc.tile_pool(name="wqtp", bufs=2 * KC // KG) as wqtp,
            tc.tile_pool(name="osb", bufs=4) as osb,
            tc.tile_pool(name="scrp", bufs=2) as scrp,
            tc.tile_pool(name="pstx", bufs=1, space="PSUM") as pstx,
            tc.tile_pool(name="pstw", bufs=2, space="PSUM") as pstw,
            tc.tile_pool(name="pop", bufs=4, space="PSUM") as pop,
            tc.tile_pool(name="psg", bufs=1, space="PSUM") as psg,
        ):
            # ---- constants ----
            ident = const.tile([128, 128], BF16)
            masks.make_identity(nc, ident[:])
            ones = const.tile([128, 128], F32)
            nc.gpsimd.memset(ones[:], 1.0)
            eps_sb = const.tile([128, 1], F32)
            nc.gpsimd.memset(eps_sb[:], NORM_EPS)
            nw_sb = const.tile([128, KC], F32)
            part = const.tile([128, WB], F32)

            # ---- DMA issues ----
            # pass-1 weight (the gamma gate) on both HWDGE queues
            wt1 = []
            for d in range(WB):
                wt = wf.tile([128, DIN], F32, tag="w")
                eng = (nc.sync, nc.scalar, nc.gpsimd)[d % 3]
                eng.dma_start(out=wt[:], in_=w_d[128 * d : 128 * (d + 1), :])
                wt1.append(wt)
            # x (f32) shares the HWDGE queues; xin bufs=2 throttles its
            # bandwidth steal from pass-1
            xts = []
            for t in range(TT):
                xt = xin.tile([128, DIN], F32)
                eng = nc.sync if t % 2 == 0 else nc.scalar
                eng.dma_start(out=xt[:], in_=x_d[128 * t : 128 * (t + 1), :])
                xts.append(xt)
            # pass-2 re-read tiles for d0..d7 (groups G2, G3). Created now
            # in d order so the wf rotation hands them the earliest-freed
            # buffers (freed by G0/G1 quantize). Even blocks issued here on
            # sync; odd blocks issued from ACT after its compute stream.
            wt2 = [
                wf.tile([128, DIN], F32, name=f"wt2_{d}", tag="w") for d in range(8)
            ]
            for d in range(0, 8, 2):
                nc.sync.dma_start(
                    out=wt2[d][:], in_=w_d[128 * d : 128 * (d + 1), :]
                )

            for k in range(KC):
                nc.gpsimd.dma_start(
                    out=nw_sb[:, k : k + 1], in_=nw_d[128 * k : 128 * (k + 1)]
                )

            def wsrc(d):
                return wt1[d] if d >= 8 else wt2[d]

            # ---- pass-1 abs partials interleaved with the early x path.
            # ACT: abs-odd blocks + per-tile Square/Sqrt; DVE: abs-even
            # blocks + bf16 casts for t0..t3. Late x tiles (t4..t7) are
            # processed post-gamma on ACT; their PE transposes thread
            # between the first matmul groups so the xin rotation delays
            # their DMA until pass-1 is off the queues. ----
            grins = []
            rmss = []
            xbs = []
            xnt = []

            def emit_abs(d):
                if d % 2 == 0:
                    nc.vector.tensor_reduce(
                        part[:, d : d + 1],
                        wt1[d][:],
                        axis=mybir.AxisListType.X,
                        op=ALU.add,
                        apply_absolute_value=True,
                    )
                else:
                    sc = scrp.tile([128, DIN], BF16, tag="scr", name=f"sc_{d}")
                    nc.scalar.activation(
                        sc[:], wt1[d][:], ACTF.Abs, accum_out=part[:, d : d + 1]
                    )

            def emit_stats(t):
                ss = spool.tile([128, 1], F32, name=f"ss_{t}")
                sq = scrp.tile([128, DIN], BF16, tag="scr", name=f"sq_{t}")
                nc.scalar.activation(
                    sq[:], xts[t][:], ACTF.Square, accum_out=ss[:]
                )
                rms = gpool.tile([128, 1], F32, name=f"rms_{t}")
                nc.scalar.activation(
                    rms[:], ss[:], ACTF.Sqrt, bias=eps_sb[:], scale=1.0 / DIN
                )
                rmss.append(rms)

            def emit_cast(t, on_act):
                xb = xbf.tile([128, DIN], BF16, name=f"xb_{t}", tag="xb")
                if on_act:
                    nc.scalar.activation(xb[:], xts[t][:], ACTF.Copy)
                else:
                    nc.vector.tensor_copy(xb[:], xts[t][:])
                xbs.append(xb)

            def emit_grin(t):
                # grin_t = gamma / rms_t: the per-token drain scale
                rinv = spool.tile([128, 1], F32, name=f"rinv_{t}")
                nc.vector.reciprocal(rinv[:], rmss[t][:])
                grin = gpool.tile([128, 1], F32, name=f"grin_{t}")
                nc.vector.tensor_scalar(
                    grin[:], rinv[:], gamma[:], None, op0=ALU.mult
                )
                grins.append(grin)

            def emit_xtrans(xb):
                xx = xntp.tile([128, KC * 128], BF16)
                xnt.append(xx)
                for g in range(KC // 4):
                    pt = pstx.tile([128, 512], BF16)
                    for i in range(4):
                        k = 4 * g + i
                        nc.tensor.transpose(
                            pt[:, 128 * i : 128 * (i + 1)],
                            xb[:, 128 * k : 128 * (k + 1)],
                            ident[:],
                        )
                    dst = xx[:, 512 * g : 512 * (g + 1)]
                    if g % 2 == 0:
                        nc.vector.tensor_copy(dst, pt[:])
                    else:
                        nc.scalar.copy(dst, pt[:])

            for i in range(4):
                emit_abs(2 * i)
                emit_abs(2 * i + 1)
                emit_stats(i)
                emit_cast(i, on_act=False)
            for d in range(8, WB):
                emit_abs(d)

            # PE: two transposes pre-gamma (PE is otherwise idle)
            emit_xtrans(xbs[0])
            emit_xtrans(xbs[1])

            # ---- gamma / tau ----
            asum = spool.tile([128, 1], F32)
            nc.vector.tensor_reduce(
                asum[:], part[:, :], axis=mybir.AxisListType.X, op=ALU.add
            )
            gps = psg.tile([128, 1], F32)
            nc.tensor.matmul(gps[:], ones[:], asum[:], start=True, stop=True)
            gamma = gpool.tile([128, 1], F32)
            nc.vector.tensor_scalar(
                gamma[:], gps[:], 1.0 / (DOUT * DIN), None, op0=ALU.mult
            )
            tau = gpool.tile([128, 1], F32)
            nc.vector.tensor_scalar(
                tau[:], gamma[:], QUANT_EPS, 0.5, op0=ALU.add, op1=ALU.mult
            )
            ntau = gpool.tile([128, 1], F32)
            nc.vector.tensor_scalar(ntau[:], tau[:], -1.0, None, op0=ALU.mult)

            # PE: t2/t3 transposes sit after the gamma matmul so the xin
            # pool rotation holds x t4..t7 DMA until pass-1 has drained
            emit_xtrans(xbs[2])
            emit_xtrans(xbs[3])

            # ACT: late x tiles + pass-2 odd-block issues interleaved
            for t in range(4, TT):
                emit_stats(t)
                emit_cast(t, on_act=True)
                dd = 2 * (t - 4) + 1  # d1, d3, d5, d7
                nc.scalar.dma_start(
                    out=wt2[dd][:], in_=w_d[128 * dd : 128 * (dd + 1), :]
                )

            # ---- quantize + transpose + matmul per 512-col group ----
            for gi, ds in enumerate(GROUPS):
                ocol = 128 * ds[0]  # output column offset of this group
                wqs = []
                for j, d in enumerate(ds):
                    src = wsrc(d)
                    pos = wmp.tile([128, DIN], BF16, tag="pos")
                    nc.vector.tensor_scalar(
                        pos[:], src[:], tau[:], None, op0=ALU.is_ge
                    )
                    neg = wmp.tile([128, DIN], BF16, tag="neg")
                    nc.vector.tensor_scalar(
                        neg[:], src[:], ntau[:], None, op0=ALU.is_le
                    )
                    wq = wqp.tile([128, DIN], BF16)
                    nc.vector.tensor_tensor(
                        wq[:], pos[:], neg[:], op=ALU.subtract
                    )
                    wqs.append(wq)
                    if gi == 0 and j < 2:
                        # interleave the tiny drain-scale ops where the
                        # rms values are already landing
                        emit_grin(2 * j)
                        emit_grin(2 * j + 1)
                wqts = []
                for kg in range(KC // KG):
                    wqt = wqtp.tile([128, KG * 512], BF16)
                    wqts.append(wqt)
                    for kk in range(KG):
                        k = KG * kg + kk
                        pt = pstw.tile([128, 512], BF16)
                        for j in range(JB):
                            nc.tensor.transpose(
                                pt[:, 128 * j : 128 * (j + 1)],
                                wqs[j][:, 128 * k : 128 * (k + 1)],
                                ident[:],
                            )
                        # fold norm_weight gain (per contraction row) in
                        dst = wqt[:, 512 * kk : 512 * (kk + 1)]
                        if gi == 0 or k % 2 == 0:
                            nc.vector.tensor_scalar(
                                dst, pt[:], nw_sb[:, k : k + 1], None,
                                op0=ALU.mult,
                            )
                        else:
                            nc.scalar.mul(dst, pt[:], nw_sb[:, k : k + 1])
                if gi == 0:
                    # t4..t7 drain scales; rms for these lands mid-phase
                    for t in range(4, TT):
                        emit_grin(t)
                late_tr = {1: 4, 3: 5, 5: 6, 6: 7} if gi == 0 else {}
                for t in range(TT):
                    if t in late_tr:
                        emit_xtrans(xbs[late_tr[t]])
                    po = pop.tile([128, 512], F32)
                    for k in range(KC):
                        nc.tensor.matmul(
                            po[:],
                            xnt[t][:, 128 * k : 128 * (k + 1)],
                            wqts[k // KG][:, 512 * (k % KG) : 512 * (k % KG + 1)],
                            start=(k == 0),
                            stop=(k == KC - 1),
                        )
                    ob = osb.tile([128, 512], F32)
                    if t % 2 == 0:
                        nc.scalar.mul(ob[:], po[:], grins[t][:])
                    else:
                        nc.vector.tensor_scalar(
                            ob[:], po[:], grins[t][:], None, op0=ALU.mult
                        )
                    oeng = (nc.gpsimd, nc.gpsimd, nc.sync, nc.scalar)[gi]
                    oeng.dma_start(
                        out=out_d[
                            128 * t : 128 * (t + 1), ocol : ocol + 512
                        ],
                        in_=ob[:],
                    )

    nc.compile()
    return nc


_cached_nc = None


def _run_traced(nc, in_maps):
    """Execute with NTFF profiling, tolerating XLA's duplicate _body
    executables (keep only the newest NTFF before conversion)."""
    import glob
    import shutil
    import tempfile

    import antenv.axon_hooks as ah
    import gauge.profiler
    from concourse import bass_utils as bu

    core_ids = list(range(NCORES))
    neff_dir = os.environ.get("BASS_KERNEL_TRACE_DIR") or tempfile.mkdtemp(
        prefix="bitlinear_prof_"
    )
    shutil.rmtree(neff_dir, ignore_errors=True)
    os.makedirs(neff_dir, exist_ok=True)

    hook = ah.get_axon_ntff_profile_hook()
    with hook(neff_dir, [0]):
        res = run_bass_kernel_spmd(nc, in_maps, core_ids=core_ids)

    ntffs = sorted(
        glob.glob(os.path.join(neff_dir, "*_body*.ntff")), key=os.path.getmtime
    )
    if not ntffs:
        print("HW exec time: unavailable (no NTFF produced)")
        return res
    for f in ntffs[:-1]:
        os.remove(f)
    profile = gauge.profiler.Profile(
        profile_path=bu.FishPath(neff_dir),
        kernel_dev_mode=True,
        profile_on_exit=False,
        bass_kernel=nc.m,
        offline_processing=True,
        fname="*_body*",
        metadata={},
    )
    pr = bu._process_ntff_profile(
        profile, neff_dir, nc, core_ids, None, False, {}, trace_events=False
    )
    if pr.exec_time_ns is not None:
        print(f"HW exec time: {pr.exec_time_ns} ns")
    return pr.as_bass_kernel_results(res.results)


def kernel(x, weight, norm_weight):
    global _cached_nc
    if _cached_nc is None:
        _cached_nc = _build()
    nc = _cached_nc

    xf = np.ascontiguousarray(
        np.asarray(x, dtype=np.float32).reshape(TOKS, DIN)
    )
    w = np.ascontiguousarray(np.asarray(weight, dtype=np.float32))
    nw = np.ascontiguousarray(np.asarray(norm_weight, dtype=np.float32))

    in_maps = []
    for c in range(NCORES):
        in_maps.append(
            {
                "x": xf[TOK * c : TOK * (c + 1)],
                "weight": w,
                "norm_weight": nw,
            }
        )

    trace = bool(os.environ.get("BASS_KERNEL_TRACE"))
    if trace:
        res = _run_traced(nc, in_maps)
    else:
        res = run_bass_kernel_spmd(nc, in_maps, core_ids=list(range(NCORES)))
    outs = [np.asarray(res.results[c]["out"]) for c in range(NCORES)]
    return np.concatenate(outs, axis=0).reshape(B, S, DOUT).astype(np.float32)
